# revision 4
# baseline (speedup 1.0000x reference)
"""GCN (2-layer, PyG GCNConv-style) on 8 Trainium2 NeuronCores via Bass/Tile.

Strategy:
  out = dinv * (A_sum @ y) + b per layer, with y = (x*dinv) @ W a node table.
  - dst nodes are split into 128-node blocks, blocks distributed over cores.
  - Edges grouped per (core, block, src-chunk); per 128-edge tile we
    dma_gather the source rows (256B each) and reduce with a one-hot matmul
    accumulating in PSUM (the segment-sum runs on the PE at full rate).
  - src-chunks of 32768 rows because dma_gather indices are int16.
  - Two launches: A = y1 build + layer-1 aggregation -> y2 table shards;
    host concatenates shards (pure data movement); B = layer-2 aggregation.
  - All heavy float math happens on device; the host only does integer
    graph preprocessing (edge sorting/partitioning) and dtype casts.
"""

import numpy as np
import ml_dtypes

import concourse.bacc as bacc
import concourse.mybir as mybir
import concourse.tile as tile
from concourse.bass_utils import run_bass_kernel_spmd

BF16 = ml_dtypes.bfloat16
P = 128

# set by test.py to collect hardware profiles
TRACE = False
LAST_EXEC_NS = []


class Cfg:
    def __init__(self, N, F_IN, HID, C_OUT, NCORES, BPC, CHUNK, SBB, SLABW):
        self.N = N
        self.F_IN = F_IN          # must be 128 (one partition load)
        self.HID = HID
        self.C_OUT = C_OUT
        self.NCORES = NCORES
        self.BPC = BPC            # dst blocks per core
        self.CHUNK = CHUNK        # gather table chunk rows (int16 reach)
        self.SBB = SBB            # blocks per superblock (gather batching)
        self.SLABW = SLABW        # xT slab width for phase 1
        self.NB = NCORES * BPC    # total blocks
        self.NPAD = self.NB * P
        assert self.NPAD >= N
        self.NCHUNKS = -(-self.NPAD // CHUNK)
        assert self.NPAD % SLABW == 0
        self.NSLAB = self.NPAD // SLABW
        # superblock layout: BPC = full SBs of SBB blocks + possibly one partial
        self.sb_sizes = []
        left = BPC
        while left > 0:
            s = min(SBB, left)
            self.sb_sizes.append(s)
            left -= s
        self.NSB = len(self.sb_sizes)


FULL = Cfg(N=100000, F_IN=128, HID=64, C_OUT=40, NCORES=8, BPC=98,
           CHUNK=32768, SBB=8, SLABW=2048)


# --------------------------------------------------------------------------
# host-side integer preprocessing
# --------------------------------------------------------------------------

def prep_edges(src, dst, cfg):
    """Group edges by (core, block, chunk); emit per-core gather-index and
    dst-local streams plus the (uniform) per-chunk tile budgets.

    Returns dict with budgets T[c], and per-core arrays IDX [NG,128,NImax//16]
    (int16) and DLOC [NSB,128,SBB*TT] (bf16), where NG = NSB*NCHUNKS.
    """
    NC, BPC, CH, SBB = cfg.NCORES, cfg.BPC, cfg.CHUNK, cfg.SBB
    nch = cfg.NCHUNKS
    core = dst // (BPC * P)
    blk = (dst % (BPC * P)) // P
    dloc = (dst % P).astype(np.int64)
    chunk = src // CH

    key = ((core * BPC + blk) * nch + chunk).astype(np.int64)
    order = np.argsort(key, kind="stable")
    skey = key[order]
    ssrc = src[order]
    sdl = dloc[order]

    nkeys = NC * BPC * nch
    counts = np.bincount(skey, minlength=nkeys)
    starts = np.zeros(nkeys + 1, np.int64)
    np.cumsum(counts, out=starts[1:])

    cnt3 = counts.reshape(NC, BPC, nch)
    T = [int(-(-cnt3[:, :, c].max() // P)) for c in range(nch)]
    T = [max(t, 1) for t in T]
    TT = int(sum(T))

    NG = cfg.NSB * nch
    nimax = SBB * max(T) * P
    IDX = np.zeros((NC, NG, P, nimax // 16), np.int16)
    DLOC = np.full((NC, cfg.NSB, P, SBB * TT), -1.0, BF16)
    offc = np.concatenate([[0], np.cumsum(T)]).astype(np.int64)

    for ci in range(NC):
        for sb in range(cfg.NSB):
            nblk = cfg.sb_sizes[sb]
            b0 = sum(cfg.sb_sizes[:sb])
            for c in range(nch):
                ni = nblk * T[c] * P
                idxs = np.zeros(ni, np.int64)
                dls = np.full(ni, -1, np.int64)
                for bl in range(nblk):
                    k = (ci * BPC + (b0 + bl)) * nch + c
                    s, n = starts[k], counts[k]
                    o = bl * T[c] * P
                    idxs[o:o + n] = ssrc[s:s + n] - c * CH
                    dls[o:o + n] = sdl[s:s + n]
                g = sb * nch + c
                wrapped = idxs.astype(np.int16).reshape(-1, 16).T  # [16, ni/16]
                IDX[ci, g, :, : ni // 16] = np.tile(wrapped, (8, 1))
                # dloc columns: [b_local, c, tt] -> col b_local*TT + offc[c] + tt
                dv = dls.reshape(nblk, T[c], P)
                for bl in range(nblk):
                    col0 = bl * TT + offc[c]
                    DLOC[ci, sb, :, col0:col0 + T[c]] = dv[bl].T.astype(BF16)
    return {"T": T, "TT": TT, "IDX": IDX, "DLOC": DLOC, "nimax": nimax}


def host_prep(x, edge_index, W1, b1, W2, b2, cfg):
    N = cfg.N
    loops = np.arange(N, dtype=np.int64)
    src = np.concatenate([edge_index[0].astype(np.int64), loops])
    dst = np.concatenate([edge_index[1].astype(np.int64), loops])

    deg = np.bincount(dst, minlength=cfg.NPAD).astype(np.float32)
    deg[deg == 0] = 1.0  # pad / isolated: dinv value never observed in output

    ep = prep_edges(src, dst, cfg)

    # xT tiled into slabs [NSLAB, 128, SLABW] bf16
    xT = np.zeros((cfg.F_IN, cfg.NPAD), np.float32)
    xT[:, :N] = x.T
    xTt = np.ascontiguousarray(
        xT.reshape(cfg.F_IN, cfg.NSLAB, cfg.SLABW).transpose(1, 0, 2)
    ).astype(BF16)

    degN = np.ascontiguousarray(deg.reshape(cfg.NB, P).T)          # [128, NB]
    degP = np.stack([degN[:, i * cfg.BPC:(i + 1) * cfg.BPC] for i in range(cfg.NCORES)])

    iota = np.broadcast_to(np.arange(P, dtype=BF16), (P, P)).copy()
    ident = np.eye(P, dtype=BF16)

    consts = {
        "xTt": xTt,
        "W1": W1.astype(BF16),
        "b1r": np.broadcast_to(b1.astype(np.float32), (P, cfg.HID)).copy(),
        "W2": W2.astype(BF16),
        "b2r": np.broadcast_to(b2.astype(np.float32), (P, cfg.C_OUT)).copy(),
        "degN": degN,
        "iota": iota,
        "ident": ident,
    }
    return ep, consts, degP


# --------------------------------------------------------------------------
# device programs
# --------------------------------------------------------------------------

def _gather_phase(nc, tc, cfg, ep, y1_ap, iota_t, body):
    """Shared gather/aggregate skeleton. body(bl_glob, matmul_feeder) where
    matmul_feeder(ph) issues the NTT one-hot matmuls into psum tile ph."""
    T, TT = ep["T"], ep["TT"]
    ncols = {c: None for c in range(cfg.NCHUNKS)}
    with (
        tc.tile_pool(name="gpool", bufs=2) as gp,
        tc.tile_pool(name="ohpool", bufs=4) as ohp,
    ):
        nidx_name = 0
        for sb in range(cfg.NSB):
            nblk = cfg.sb_sizes[sb]
            b0 = sum(cfg.sb_sizes[:sb])
            g_ts = []
            for c in range(cfg.NCHUNKS):
                ni = nblk * T[c] * P
                g = sb * cfg.NCHUNKS + c
                idx_t = gp.tile([P, ni // 16], mybir.dt.int16, tag=f"idx{c}")
                nc.sync.dma_start(out=idx_t[:], in_=nc.t_IDX[g, :, : ni // 16])
                gt = gp.tile([P, nblk * T[c], P], mybir.dt.bfloat16, tag=f"g{c}")
                lo = c * cfg.CHUNK
                hi = min(lo + cfg.CHUNK, cfg.NPAD)
                nc.gpsimd.dma_gather(
                    out_ap=gt[:],
                    in_ap=y1_ap[lo:hi, :],
                    idxs_ap=idx_t[:],
                    num_idxs=ni,
                    num_idxs_reg=ni,
                    elem_size=P,
                    single_packet=False,
                )
                g_ts.append(gt)
            dloc_t = gp.tile([P, cfg.SBB * TT], mybir.dt.bfloat16, tag="dloc")
            nc.sync.dma_start(
                out=dloc_t[:, : nblk * TT], in_=nc.t_DLOC[sb, :, : nblk * TT]
            )

            for bl in range(nblk):
                def feeder(ph, rhs_w):
                    k = 0
                    for c in range(cfg.NCHUNKS):
                        for tt in range(T[c]):
                            col = bl * TT + sum(T[:c]) + tt
                            oh = ohp.tile([P, P], mybir.dt.bfloat16, tag="oh")
                            nc.vector.tensor_tensor(
                                out=oh[:],
                                in0=dloc_t[:, col:col + 1].to_broadcast([P, P]),
                                in1=iota_t[:],
                                op=mybir.AluOpType.is_equal,
                            )
                            nc.tensor.matmul(
                                out=ph[:],
                                lhsT=oh[:],
                                rhs=g_ts[c][:, bl * T[c] + tt, 0:rhs_w],
                                start=(k == 0),
                                stop=(k == TT - 1),
                            )
                            k += 1
                body(b0 + bl, feeder)


def build_launch_A(cfg, ep):
    nc = bacc.Bacc(None, target_bir_lowering=False, name="gcn_a")
    HID, COUT = cfg.HID, cfg.C_OUT
    t_xTt = nc.dram_tensor("xTt", [cfg.NSLAB, P, cfg.SLABW], mybir.dt.bfloat16, kind="ExternalInput")
    t_W1 = nc.dram_tensor("W1", [cfg.F_IN, HID], mybir.dt.bfloat16, kind="ExternalInput")
    t_b1r = nc.dram_tensor("b1r", [P, HID], mybir.dt.float32, kind="ExternalInput")
    t_W2 = nc.dram_tensor("W2", [HID, COUT], mybir.dt.bfloat16, kind="ExternalInput")
    t_degN = nc.dram_tensor("degN", [P, cfg.NB], mybir.dt.float32, kind="ExternalInput")
    t_degP = nc.dram_tensor("degP", [P, cfg.BPC], mybir.dt.float32, kind="ExternalInput")
    t_iota = nc.dram_tensor("iota", [P, P], mybir.dt.bfloat16, kind="ExternalInput")
    t_ident = nc.dram_tensor("ident", [P, P], mybir.dt.bfloat16, kind="ExternalInput")
    nc.t_IDX = nc.dram_tensor("IDX", list(ep["IDX"].shape[1:]), mybir.dt.int16, kind="ExternalInput")
    nc.t_DLOC = nc.dram_tensor("DLOC", list(ep["DLOC"].shape[1:]), mybir.dt.bfloat16, kind="ExternalInput")
    t_y2s = nc.dram_tensor("y2s", [cfg.BPC * P, P], mybir.dt.bfloat16, kind="ExternalOutput")

    with tile.TileContext(nc) as tc:
        with (
            tc.tile_pool(name="consts", bufs=1) as cp,
            tc.tile_pool(name="dram", bufs=1, space="DRAM") as dp,
        ):
            y1 = dp.tile([cfg.NPAD, P], mybir.dt.bfloat16)
            w1_t = cp.tile([cfg.F_IN, HID], mybir.dt.bfloat16)
            nc.sync.dma_start(out=w1_t[:], in_=t_W1[:, :])
            w2_t = cp.tile([HID, COUT], mybir.dt.bfloat16)
            nc.sync.dma_start(out=w2_t[:], in_=t_W2[:, :])
            b1r_t = cp.tile([P, HID], mybir.dt.float32)
            nc.sync.dma_start(out=b1r_t[:], in_=t_b1r[:, :])
            iota_t = cp.tile([P, P], mybir.dt.bfloat16)
            nc.sync.dma_start(out=iota_t[:], in_=t_iota[:, :])
            ident_t = cp.tile([P, P], mybir.dt.bfloat16)
            nc.sync.dma_start(out=ident_t[:], in_=t_ident[:, :])

            # dinv tables: 1/sqrt(deg)
            degN_t = cp.tile([P, cfg.NB], mybir.dt.float32)
            nc.sync.dma_start(out=degN_t[:], in_=t_degN[:, :])
            sqN = cp.tile([P, cfg.NB], mybir.dt.float32)
            nc.scalar.activation(out=sqN[:], in_=degN_t[:], func=mybir.ActivationFunctionType.Sqrt)
            dinvN = cp.tile([P, cfg.NB], mybir.dt.float32)
            nc.vector.reciprocal(out=dinvN[:], in_=sqN[:])
            degP_t = cp.tile([P, cfg.BPC], mybir.dt.float32)
            nc.sync.dma_start(out=degP_t[:], in_=t_degP[:, :])
            sqP = cp.tile([P, cfg.BPC], mybir.dt.float32)
            nc.scalar.activation(out=sqP[:], in_=degP_t[:], func=mybir.ActivationFunctionType.Sqrt)
            dinvP = cp.tile([P, cfg.BPC], mybir.dt.float32)
            nc.vector.reciprocal(out=dinvP[:], in_=sqP[:])

            # phase 1: y1 = (x @ W1) * dinv  (bf16 rows padded to 128 elems)
            npc = cfg.SLABW // P  # node chunks per slab
            with (
                tc.tile_pool(name="ph1", bufs=3) as p1,
                tc.tile_pool(name="ph1ps", bufs=2, space="PSUM") as p1p,
            ):
                for s in range(cfg.NSLAB):
                    slab = p1.tile([P, cfg.SLABW], mybir.dt.bfloat16, tag="slab")
                    nc.sync.dma_start(out=slab[:], in_=t_xTt[s, :, :])
                    for j in range(npc):
                        jj = s * npc + j
                        ps = p1p.tile([P, HID], mybir.dt.float32, tag="psy")
                        nc.tensor.matmul(
                            out=ps[:], lhsT=slab[:, j * P:(j + 1) * P], rhs=w1_t[:],
                            start=True, stop=True,
                        )
                        row = p1.tile([P, P], mybir.dt.bfloat16, tag="row")
                        nc.vector.memset(row[:, HID:], 0)
                        nc.vector.tensor_tensor(
                            out=row[:, :HID], in0=ps[:],
                            in1=dinvN[:, jj:jj + 1].to_broadcast([P, HID]),
                            op=mybir.AluOpType.mult,
                        )
                        nc.sync.dma_start(out=y1[jj * P:(jj + 1) * P, :], in_=row[:])

            # phase 2: layer-1 aggregation + y2 table rows
            with (
                tc.tile_pool(name="ep1", bufs=3) as e1,
                tc.tile_pool(name="hps", bufs=2, space="PSUM") as hps,
                tc.tile_pool(name="tps", bufs=2, space="PSUM") as tps,
                tc.tile_pool(name="yps", bufs=2, space="PSUM") as yps,
            ):
                def body(bg, feeder):
                    ph = hps.tile([P, HID], mybir.dt.float32, tag="ph")
                    feeder(ph, HID)
                    dv = dinvP[:, bg:bg + 1]
                    t1 = e1.tile([P, HID], mybir.dt.float32, tag="t1")
                    nc.vector.tensor_tensor(
                        out=t1[:], in0=ph[:], in1=dv.to_broadcast([P, HID]),
                        op=mybir.AluOpType.mult,
                    )
                    t2 = e1.tile([P, HID], mybir.dt.float32, tag="t2")
                    nc.vector.tensor_tensor(
                        out=t2[:], in0=t1[:], in1=b1r_t[:], op=mybir.AluOpType.add,
                    )
                    hd = e1.tile([P, HID], mybir.dt.bfloat16, tag="hd")
                    nc.scalar.activation(
                        out=hd[:], in_=t2[:],
                        func=mybir.ActivationFunctionType.Relu, scale=dv,
                    )
                    ptr = tps.tile([HID, P], mybir.dt.bfloat16, tag="ptr")
                    nc.tensor.transpose(out=ptr[:], in_=hd[:], identity=ident_t[:])
                    hdT = e1.tile([HID, P], mybir.dt.bfloat16, tag="hdT")
                    nc.vector.tensor_copy(out=hdT[:], in_=ptr[:])
                    py2 = yps.tile([P, COUT], mybir.dt.float32, tag="py2")
                    nc.tensor.matmul(out=py2[:], lhsT=hdT[:], rhs=w2_t[:], start=True, stop=True)
                    yrow = e1.tile([P, P], mybir.dt.bfloat16, tag="yrow")
                    nc.vector.memset(yrow[:, COUT:], 0)
                    nc.vector.tensor_copy(out=yrow[:, :COUT], in_=py2[:])
                    nc.sync.dma_start(out=t_y2s[bg * P:(bg + 1) * P, :], in_=yrow[:])

                _gather_phase(nc, tc, cfg, ep, y1[:, :], iota_t, body)
    nc.compile()
    return nc


def build_launch_B(cfg, ep):
    nc = bacc.Bacc(None, target_bir_lowering=False, name="gcn_b")
    COUT = cfg.C_OUT
    t_y2 = nc.dram_tensor("y2", [cfg.NPAD, P], mybir.dt.bfloat16, kind="ExternalInput")
    t_b2r = nc.dram_tensor("b2r", [P, COUT], mybir.dt.float32, kind="ExternalInput")
    t_degP = nc.dram_tensor("degP", [P, cfg.BPC], mybir.dt.float32, kind="ExternalInput")
    t_iota = nc.dram_tensor("iota", [P, P], mybir.dt.bfloat16, kind="ExternalInput")
    nc.t_IDX = nc.dram_tensor("IDX", list(ep["IDX"].shape[1:]), mybir.dt.int16, kind="ExternalInput")
    nc.t_DLOC = nc.dram_tensor("DLOC", list(ep["DLOC"].shape[1:]), mybir.dt.bfloat16, kind="ExternalInput")
    t_out = nc.dram_tensor("outs", [cfg.BPC * P, COUT], mybir.dt.float32, kind="ExternalOutput")

    with tile.TileContext(nc) as tc:
        with tc.tile_pool(name="consts", bufs=1) as cp:
            iota_t = cp.tile([P, P], mybir.dt.bfloat16)
            nc.sync.dma_start(out=iota_t[:], in_=t_iota[:, :])
            b2r_t = cp.tile([P, COUT], mybir.dt.float32)
            nc.sync.dma_start(out=b2r_t[:], in_=t_b2r[:, :])
            degP_t = cp.tile([P, cfg.BPC], mybir.dt.float32)
            nc.sync.dma_start(out=degP_t[:], in_=t_degP[:, :])
            sqP = cp.tile([P, cfg.BPC], mybir.dt.float32)
            nc.scalar.activation(out=sqP[:], in_=degP_t[:], func=mybir.ActivationFunctionType.Sqrt)
            dinvP = cp.tile([P, cfg.BPC], mybir.dt.float32)
            nc.vector.reciprocal(out=dinvP[:], in_=sqP[:])

            with (
                tc.tile_pool(name="ep2", bufs=3) as e2,
                tc.tile_pool(name="ops", bufs=2, space="PSUM") as ops,
            ):
                def body(bg, feeder):
                    po = ops.tile([P, COUT], mybir.dt.float32, tag="po")
                    feeder(po, COUT)
                    t1 = e2.tile([P, COUT], mybir.dt.float32, tag="t1")
                    nc.vector.tensor_tensor(
                        out=t1[:], in0=po[:],
                        in1=dinvP[:, bg:bg + 1].to_broadcast([P, COUT]),
                        op=mybir.AluOpType.mult,
                    )
                    ot = e2.tile([P, COUT], mybir.dt.float32, tag="ot")
                    nc.vector.tensor_tensor(
                        out=ot[:], in0=t1[:], in1=b2r_t[:], op=mybir.AluOpType.add,
                    )
                    nc.sync.dma_start(out=t_out[bg * P:(bg + 1) * P, :], in_=ot[:])

                _gather_phase(nc, tc, cfg, ep, t_y2[:, :], iota_t, body)
    nc.compile()
    return nc


# --------------------------------------------------------------------------
# entry point
# --------------------------------------------------------------------------

def run(x, edge_index, W1, b1, W2, b2, cfg, runner=None):
    global LAST_EXEC_NS
    LAST_EXEC_NS = []
    ep, consts, degP = host_prep(
        np.asarray(x, np.float32), np.asarray(edge_index), np.asarray(W1),
        np.asarray(b1), np.asarray(W2), np.asarray(b2), cfg)

    ncA = build_launch_A(cfg, ep)
    ncB = build_launch_B(cfg, ep)

    in_A = []
    for ci in range(cfg.NCORES):
        m = {k: consts[k] for k in
             ("xTt", "W1", "b1r", "W2", "degN", "iota", "ident")}
        m["degP"] = degP[ci]
        m["IDX"] = ep["IDX"][ci]
        m["DLOC"] = ep["DLOC"][ci]
        in_A.append(m)

    if runner is None:
        def runner(nc, in_maps):
            res = run_bass_kernel_spmd(
                nc, in_maps, core_ids=list(range(cfg.NCORES)), trace=TRACE)
            LAST_EXEC_NS.append(res.exec_time_ns)
            return res.results

    resA = runner(ncA, in_A)
    y2_full = np.concatenate([r["y2s"] for r in resA], axis=0)  # [NPAD, 128]

    in_B = []
    for ci in range(cfg.NCORES):
        m = {
            "y2": y2_full,
            "b2r": consts["b2r"],
            "iota": consts["iota"],
            "degP": degP[ci],
            "IDX": ep["IDX"][ci],
            "DLOC": ep["DLOC"][ci],
        }
        in_B.append(m)
    resB = runner(ncB, in_B)
    out = np.concatenate([r["outs"] for r in resB], axis=0)  # [NPAD, C_OUT]
    return out[: cfg.N]


def kernel(x, edge_index, W1, b1, W2, b2):
    return run(x, edge_index, W1, b1, W2, b2, FULL)


# revision 6
# speedup vs baseline: 1.2450x; 1.2450x over previous
"""GCN (2-layer, PyG GCNConv-style) on 8 Trainium2 NeuronCores via Bass/Tile.

Strategy:
  out = dinv * (A_sum @ y) + b per layer, with y = (x*dinv) @ W a node table.
  - dst nodes are split into 128-node blocks, blocks distributed over cores.
  - Edges grouped per (core, block, src-chunk); per 128-edge tile we
    dma_gather the source rows (256B each) and reduce with a one-hot matmul
    accumulating in PSUM (the segment-sum runs on the PE at full rate).
  - src-chunks of 32768 rows because dma_gather indices are int16.
  - Two launches: A = y1 build + layer-1 aggregation -> y2 table shards;
    host concatenates shards (pure data movement); B = layer-2 aggregation.
  - All heavy float math happens on device; the host only does integer
    graph preprocessing (edge sorting/partitioning) and dtype casts.
"""

import numpy as np
import ml_dtypes

import concourse.bacc as bacc
import concourse.mybir as mybir
import concourse.tile as tile
from concourse.bass_utils import run_bass_kernel_spmd

BF16 = ml_dtypes.bfloat16
P = 128

# set by test.py to collect hardware profiles
TRACE = False
# emit pad-zeroing memsets (needed only to satisfy the simulator's
# uninitialized-read checker; the padded bytes are never used by compute)
SIM_SAFE = False
LAST_EXEC_NS = []


class Cfg:
    def __init__(self, N, F_IN, HID, C_OUT, NCORES, BPC, CHUNK, SBB, SLABW):
        self.N = N
        self.F_IN = F_IN          # must be 128 (one partition load)
        self.HID = HID
        self.C_OUT = C_OUT
        self.NCORES = NCORES
        self.BPC = BPC            # dst blocks per core
        self.CHUNK = CHUNK        # gather table chunk rows (int16 reach)
        self.SBB = SBB            # blocks per superblock (gather batching)
        self.SLABW = SLABW        # xT slab width for phase 1
        self.NB = NCORES * BPC    # total blocks
        self.NPAD = self.NB * P
        assert self.NPAD >= N
        self.NCHUNKS = -(-self.NPAD // CHUNK)
        assert self.NPAD % SLABW == 0
        self.NSLAB = self.NPAD // SLABW
        # superblock layout: BPC = full SBs of SBB blocks + possibly one partial
        self.sb_sizes = []
        left = BPC
        while left > 0:
            s = min(SBB, left)
            self.sb_sizes.append(s)
            left -= s
        self.NSB = len(self.sb_sizes)


FULL = Cfg(N=100000, F_IN=128, HID=64, C_OUT=40, NCORES=8, BPC=98,
           CHUNK=32768, SBB=8, SLABW=2048)


# --------------------------------------------------------------------------
# host-side integer preprocessing
# --------------------------------------------------------------------------

def prep_edges(src, dst, cfg):
    """Group edges by (core, block, chunk); emit per-core gather-index and
    dst-local streams plus the (uniform) per-chunk tile budgets.

    Returns dict with budgets T[c], and per-core arrays IDX [NG,128,NImax//16]
    (int16) and DLOC [NSB,128,SBB*TT] (bf16), where NG = NSB*NCHUNKS.
    """
    NC, BPC, CH, SBB = cfg.NCORES, cfg.BPC, cfg.CHUNK, cfg.SBB
    nch = cfg.NCHUNKS
    core = dst // (BPC * P)
    blk = (dst % (BPC * P)) // P
    dloc = (dst % P).astype(np.int64)
    chunk = src // CH

    key = ((core * BPC + blk) * nch + chunk).astype(np.int64)
    order = np.argsort(key, kind="stable")
    skey = key[order]
    ssrc = src[order]
    sdl = dloc[order]

    nkeys = NC * BPC * nch
    counts = np.bincount(skey, minlength=nkeys)
    starts = np.zeros(nkeys + 1, np.int64)
    np.cumsum(counts, out=starts[1:])

    cnt3 = counts.reshape(NC, BPC, nch)
    T = [int(-(-cnt3[:, :, c].max() // P)) for c in range(nch)]
    T = [max(t, 1) for t in T]
    TT = int(sum(T))

    NG = cfg.NSB * nch
    nimax = SBB * max(T) * P
    IDX = np.zeros((NC, NG, P, nimax // 16), np.int16)
    DLOC = np.full((NC, cfg.NSB, P, SBB * TT), -1.0, BF16)
    offc = np.concatenate([[0], np.cumsum(T)]).astype(np.int64)

    for ci in range(NC):
        for sb in range(cfg.NSB):
            nblk = cfg.sb_sizes[sb]
            b0 = sum(cfg.sb_sizes[:sb])
            for c in range(nch):
                ni = nblk * T[c] * P
                idxs = np.zeros(ni, np.int64)
                dls = np.full(ni, -1, np.int64)
                for bl in range(nblk):
                    k = (ci * BPC + (b0 + bl)) * nch + c
                    s, n = starts[k], counts[k]
                    o = bl * T[c] * P
                    idxs[o:o + n] = ssrc[s:s + n] - c * CH
                    dls[o:o + n] = sdl[s:s + n]
                g = sb * nch + c
                wrapped = idxs.astype(np.int16).reshape(-1, 16).T  # [16, ni/16]
                IDX[ci, g, :, : ni // 16] = np.tile(wrapped, (8, 1))
                # dloc columns: [b_local, c, tt] -> col b_local*TT + offc[c] + tt
                dv = dls.reshape(nblk, T[c], P)
                for bl in range(nblk):
                    col0 = bl * TT + offc[c]
                    DLOC[ci, sb, :, col0:col0 + T[c]] = dv[bl].T.astype(BF16)
    return {"T": T, "TT": TT, "IDX": IDX, "DLOC": DLOC, "nimax": nimax}


def host_prep(x, edge_index, W1, b1, W2, b2, cfg):
    N = cfg.N
    loops = np.arange(N, dtype=np.int64)
    src = np.concatenate([edge_index[0].astype(np.int64), loops])
    dst = np.concatenate([edge_index[1].astype(np.int64), loops])

    deg = np.bincount(dst, minlength=cfg.NPAD).astype(np.float32)
    deg[deg == 0] = 1.0  # pad / isolated: dinv value never observed in output

    ep = prep_edges(src, dst, cfg)

    # xT tiled into slabs [NSLAB, 128, SLABW] bf16
    xT = np.zeros((cfg.F_IN, cfg.NPAD), np.float32)
    xT[:, :N] = x.T
    xTt = np.ascontiguousarray(
        xT.reshape(cfg.F_IN, cfg.NSLAB, cfg.SLABW).transpose(1, 0, 2)
    ).astype(BF16)

    degN = np.ascontiguousarray(deg.reshape(cfg.NB, P).T)          # [128, NB]
    degP = np.stack([degN[:, i * cfg.BPC:(i + 1) * cfg.BPC] for i in range(cfg.NCORES)])

    iota = np.broadcast_to(np.arange(P, dtype=BF16), (P, P)).copy()
    ident = np.eye(P, dtype=BF16)

    consts = {
        "xTt": xTt,
        "W1": W1.astype(BF16),
        "b1r": np.broadcast_to(b1.astype(np.float32), (P, cfg.HID)).copy(),
        "W2": W2.astype(BF16),
        "b2r": np.broadcast_to(b2.astype(np.float32), (P, cfg.C_OUT)).copy(),
        "degN": degN,
        "iota": iota,
        "ident": ident,
    }
    return ep, consts, degP


# --------------------------------------------------------------------------
# device programs
# --------------------------------------------------------------------------

def _gather_phase(nc, tc, cfg, ep, y1_ap, iota_t, body):
    """Shared gather/aggregate skeleton. body(bl_glob, matmul_feeder) where
    matmul_feeder(ph) issues the NTT one-hot matmuls into psum tile ph."""
    T, TT = ep["T"], ep["TT"]
    ncols = {c: None for c in range(cfg.NCHUNKS)}
    with (
        tc.tile_pool(name="gpool", bufs=2) as gp,
        tc.tile_pool(name="ohpool", bufs=4) as ohp,
    ):
        nidx_name = 0
        for sb in range(cfg.NSB):
            nblk = cfg.sb_sizes[sb]
            b0 = sum(cfg.sb_sizes[:sb])
            g_ts = []
            for c in range(cfg.NCHUNKS):
                ni = nblk * T[c] * P
                g = sb * cfg.NCHUNKS + c
                idx_t = gp.tile([P, ni // 16], mybir.dt.int16, tag=f"idx{c}")
                nc.sync.dma_start(out=idx_t[:], in_=nc.t_IDX[g, :, : ni // 16])
                gt = gp.tile([P, nblk * T[c], P], mybir.dt.bfloat16, tag=f"g{c}")
                lo = c * cfg.CHUNK
                hi = min(lo + cfg.CHUNK, cfg.NPAD)
                nc.gpsimd.dma_gather(
                    out_ap=gt[:],
                    in_ap=y1_ap[lo:hi, :],
                    idxs_ap=idx_t[:],
                    num_idxs=ni,
                    num_idxs_reg=ni,
                    elem_size=P,
                    single_packet=False,
                    queue_num=c % 4,
                )
                g_ts.append(gt)
            dloc_t = gp.tile([P, cfg.SBB * TT], mybir.dt.bfloat16, tag="dloc")
            nc.sync.dma_start(
                out=dloc_t[:, : nblk * TT], in_=nc.t_DLOC[sb, :, : nblk * TT]
            )

            for bl in range(nblk):
                def feeder(ph, rhs_w):
                    k = 0
                    for c in range(cfg.NCHUNKS):
                        for tt in range(T[c]):
                            col = bl * TT + sum(T[:c]) + tt
                            oh = ohp.tile([P, P], mybir.dt.bfloat16, tag="oh")
                            nc.vector.tensor_tensor(
                                out=oh[:],
                                in0=dloc_t[:, col:col + 1].to_broadcast([P, P]),
                                in1=iota_t[:],
                                op=mybir.AluOpType.is_equal,
                            )
                            nc.tensor.matmul(
                                out=ph[:],
                                lhsT=oh[:],
                                rhs=g_ts[c][:, bl * T[c] + tt, 0:rhs_w],
                                start=(k == 0),
                                stop=(k == TT - 1),
                            )
                            k += 1
                body(b0 + bl, feeder)


def build_launch_A(cfg, ep):
    nc = bacc.Bacc(None, target_bir_lowering=False, name="gcn_a", num_swdge_queues=4)
    HID, COUT = cfg.HID, cfg.C_OUT
    t_xTt = nc.dram_tensor("xTt", [cfg.NSLAB, P, cfg.SLABW], mybir.dt.bfloat16, kind="ExternalInput")
    t_W1 = nc.dram_tensor("W1", [cfg.F_IN, HID], mybir.dt.bfloat16, kind="ExternalInput")
    t_b1r = nc.dram_tensor("b1r", [P, HID], mybir.dt.float32, kind="ExternalInput")
    t_W2 = nc.dram_tensor("W2", [HID, COUT], mybir.dt.bfloat16, kind="ExternalInput")
    t_degN = nc.dram_tensor("degN", [P, cfg.NB], mybir.dt.float32, kind="ExternalInput")
    t_degP = nc.dram_tensor("degP", [P, cfg.BPC], mybir.dt.float32, kind="ExternalInput")
    t_iota = nc.dram_tensor("iota", [P, P], mybir.dt.bfloat16, kind="ExternalInput")
    t_ident = nc.dram_tensor("ident", [P, P], mybir.dt.bfloat16, kind="ExternalInput")
    nc.t_IDX = nc.dram_tensor("IDX", list(ep["IDX"].shape[1:]), mybir.dt.int16, kind="ExternalInput")
    nc.t_DLOC = nc.dram_tensor("DLOC", list(ep["DLOC"].shape[1:]), mybir.dt.bfloat16, kind="ExternalInput")
    t_y2s = nc.dram_tensor("y2s", [cfg.BPC * P, P], mybir.dt.bfloat16, kind="ExternalOutput")

    with tile.TileContext(nc) as tc:
        with (
            tc.tile_pool(name="consts", bufs=1) as cp,
            tc.tile_pool(name="dram", bufs=1, space="DRAM") as dp,
        ):
            y1 = dp.tile([cfg.NPAD, P], mybir.dt.bfloat16)
            w1_t = cp.tile([cfg.F_IN, HID], mybir.dt.bfloat16)
            nc.sync.dma_start(out=w1_t[:], in_=t_W1[:, :])
            w2_t = cp.tile([HID, COUT], mybir.dt.bfloat16)
            nc.sync.dma_start(out=w2_t[:], in_=t_W2[:, :])
            b1r_t = cp.tile([P, HID], mybir.dt.float32)
            nc.sync.dma_start(out=b1r_t[:], in_=t_b1r[:, :])
            iota_t = cp.tile([P, P], mybir.dt.bfloat16)
            nc.sync.dma_start(out=iota_t[:], in_=t_iota[:, :])
            ident_t = cp.tile([P, P], mybir.dt.bfloat16)
            nc.sync.dma_start(out=ident_t[:], in_=t_ident[:, :])

            # dinv tables: 1/sqrt(deg)
            degN_t = cp.tile([P, cfg.NB], mybir.dt.float32)
            nc.sync.dma_start(out=degN_t[:], in_=t_degN[:, :])
            sqN = cp.tile([P, cfg.NB], mybir.dt.float32)
            nc.scalar.activation(out=sqN[:], in_=degN_t[:], func=mybir.ActivationFunctionType.Sqrt)
            dinvN = cp.tile([P, cfg.NB], mybir.dt.float32)
            nc.vector.reciprocal(out=dinvN[:], in_=sqN[:])
            degP_t = cp.tile([P, cfg.BPC], mybir.dt.float32)
            nc.sync.dma_start(out=degP_t[:], in_=t_degP[:, :])
            sqP = cp.tile([P, cfg.BPC], mybir.dt.float32)
            nc.scalar.activation(out=sqP[:], in_=degP_t[:], func=mybir.ActivationFunctionType.Sqrt)
            dinvP = cp.tile([P, cfg.BPC], mybir.dt.float32)
            nc.vector.reciprocal(out=dinvP[:], in_=sqP[:])

            # phase 1: y1 = (x @ W1) * dinv  (bf16 rows padded to 128 elems)
            npc = cfg.SLABW // P  # node chunks per slab
            with (
                tc.tile_pool(name="ph1", bufs=3) as p1,
                tc.tile_pool(name="ph1ps", bufs=2, space="PSUM") as p1p,
            ):
                for s in range(cfg.NSLAB):
                    slab = p1.tile([P, cfg.SLABW], mybir.dt.bfloat16, tag="slab")
                    nc.sync.dma_start(out=slab[:], in_=t_xTt[s, :, :])
                    for j in range(npc):
                        jj = s * npc + j
                        ps = p1p.tile([P, HID], mybir.dt.float32, tag="psy")
                        nc.tensor.matmul(
                            out=ps[:], lhsT=slab[:, j * P:(j + 1) * P], rhs=w1_t[:],
                            start=True, stop=True,
                        )
                        row = p1.tile([P, P], mybir.dt.bfloat16, tag="row")
                        if SIM_SAFE:
                            nc.vector.memset(row[:, HID:], 0)
                        nc.vector.tensor_tensor(
                            out=row[:, :HID], in0=ps[:],
                            in1=dinvN[:, jj:jj + 1].to_broadcast([P, HID]),
                            op=mybir.AluOpType.mult,
                        )
                        nc.sync.dma_start(out=y1[jj * P:(jj + 1) * P, :], in_=row[:])

            # phase 2: layer-1 aggregation + y2 table rows
            with (
                tc.tile_pool(name="ep1", bufs=3) as e1,
                tc.tile_pool(name="hps", bufs=2, space="PSUM") as hps,
                tc.tile_pool(name="tps", bufs=2, space="PSUM") as tps,
                tc.tile_pool(name="yps", bufs=2, space="PSUM") as yps,
            ):
                def body(bg, feeder):
                    ph = hps.tile([P, HID], mybir.dt.float32, tag="ph")
                    feeder(ph, HID)
                    dv = dinvP[:, bg:bg + 1]
                    t1 = e1.tile([P, HID], mybir.dt.float32, tag="t1")
                    nc.vector.tensor_tensor(
                        out=t1[:], in0=ph[:], in1=dv.to_broadcast([P, HID]),
                        op=mybir.AluOpType.mult,
                    )
                    t2 = e1.tile([P, HID], mybir.dt.float32, tag="t2")
                    nc.vector.tensor_tensor(
                        out=t2[:], in0=t1[:], in1=b1r_t[:], op=mybir.AluOpType.add,
                    )
                    hd = e1.tile([P, HID], mybir.dt.bfloat16, tag="hd")
                    nc.scalar.activation(
                        out=hd[:], in_=t2[:],
                        func=mybir.ActivationFunctionType.Relu, scale=dv,
                    )
                    ptr = tps.tile([HID, P], mybir.dt.bfloat16, tag="ptr")
                    nc.tensor.transpose(out=ptr[:], in_=hd[:], identity=ident_t[:])
                    hdT = e1.tile([HID, P], mybir.dt.bfloat16, tag="hdT")
                    nc.vector.tensor_copy(out=hdT[:], in_=ptr[:])
                    py2 = yps.tile([P, COUT], mybir.dt.float32, tag="py2")
                    nc.tensor.matmul(out=py2[:], lhsT=hdT[:], rhs=w2_t[:], start=True, stop=True)
                    yrow = e1.tile([P, P], mybir.dt.bfloat16, tag="yrow")
                    if SIM_SAFE:
                        nc.vector.memset(yrow[:, COUT:], 0)
                    nc.vector.tensor_copy(out=yrow[:, :COUT], in_=py2[:])
                    nc.sync.dma_start(out=t_y2s[bg * P:(bg + 1) * P, :], in_=yrow[:])

                _gather_phase(nc, tc, cfg, ep, y1[:, :], iota_t, body)
    nc.compile()
    return nc


def build_launch_B(cfg, ep):
    nc = bacc.Bacc(None, target_bir_lowering=False, name="gcn_b", num_swdge_queues=4)
    COUT = cfg.C_OUT
    t_y2 = nc.dram_tensor("y2", [cfg.NPAD, P], mybir.dt.bfloat16, kind="ExternalInput")
    t_b2r = nc.dram_tensor("b2r", [P, COUT], mybir.dt.float32, kind="ExternalInput")
    t_degP = nc.dram_tensor("degP", [P, cfg.BPC], mybir.dt.float32, kind="ExternalInput")
    t_iota = nc.dram_tensor("iota", [P, P], mybir.dt.bfloat16, kind="ExternalInput")
    nc.t_IDX = nc.dram_tensor("IDX", list(ep["IDX"].shape[1:]), mybir.dt.int16, kind="ExternalInput")
    nc.t_DLOC = nc.dram_tensor("DLOC", list(ep["DLOC"].shape[1:]), mybir.dt.bfloat16, kind="ExternalInput")
    t_out = nc.dram_tensor("outs", [cfg.BPC * P, COUT], mybir.dt.float32, kind="ExternalOutput")

    with tile.TileContext(nc) as tc:
        with tc.tile_pool(name="consts", bufs=1) as cp:
            iota_t = cp.tile([P, P], mybir.dt.bfloat16)
            nc.sync.dma_start(out=iota_t[:], in_=t_iota[:, :])
            b2r_t = cp.tile([P, COUT], mybir.dt.float32)
            nc.sync.dma_start(out=b2r_t[:], in_=t_b2r[:, :])
            degP_t = cp.tile([P, cfg.BPC], mybir.dt.float32)
            nc.sync.dma_start(out=degP_t[:], in_=t_degP[:, :])
            sqP = cp.tile([P, cfg.BPC], mybir.dt.float32)
            nc.scalar.activation(out=sqP[:], in_=degP_t[:], func=mybir.ActivationFunctionType.Sqrt)
            dinvP = cp.tile([P, cfg.BPC], mybir.dt.float32)
            nc.vector.reciprocal(out=dinvP[:], in_=sqP[:])

            with (
                tc.tile_pool(name="ep2", bufs=3) as e2,
                tc.tile_pool(name="ops", bufs=2, space="PSUM") as ops,
            ):
                def body(bg, feeder):
                    po = ops.tile([P, COUT], mybir.dt.float32, tag="po")
                    feeder(po, COUT)
                    t1 = e2.tile([P, COUT], mybir.dt.float32, tag="t1")
                    nc.vector.tensor_tensor(
                        out=t1[:], in0=po[:],
                        in1=dinvP[:, bg:bg + 1].to_broadcast([P, COUT]),
                        op=mybir.AluOpType.mult,
                    )
                    ot = e2.tile([P, COUT], mybir.dt.float32, tag="ot")
                    nc.vector.tensor_tensor(
                        out=ot[:], in0=t1[:], in1=b2r_t[:], op=mybir.AluOpType.add,
                    )
                    nc.sync.dma_start(out=t_out[bg * P:(bg + 1) * P, :], in_=ot[:])

                _gather_phase(nc, tc, cfg, ep, t_y2[:, :], iota_t, body)
    nc.compile()
    return nc


# --------------------------------------------------------------------------
# entry point
# --------------------------------------------------------------------------

def run(x, edge_index, W1, b1, W2, b2, cfg, runner=None):
    global LAST_EXEC_NS
    LAST_EXEC_NS = []
    ep, consts, degP = host_prep(
        np.asarray(x, np.float32), np.asarray(edge_index), np.asarray(W1),
        np.asarray(b1), np.asarray(W2), np.asarray(b2), cfg)

    ncA = build_launch_A(cfg, ep)
    ncB = build_launch_B(cfg, ep)

    in_A = []
    for ci in range(cfg.NCORES):
        m = {k: consts[k] for k in
             ("xTt", "W1", "b1r", "W2", "degN", "iota", "ident")}
        m["degP"] = degP[ci]
        m["IDX"] = ep["IDX"][ci]
        m["DLOC"] = ep["DLOC"][ci]
        in_A.append(m)

    if runner is None:
        def runner(nc, in_maps):
            res = run_bass_kernel_spmd(
                nc, in_maps, core_ids=list(range(cfg.NCORES)), trace=TRACE)
            LAST_EXEC_NS.append(res.exec_time_ns)
            return res.results

    resA = runner(ncA, in_A)
    y2_full = np.concatenate([r["y2s"] for r in resA], axis=0)  # [NPAD, 128]

    in_B = []
    for ci in range(cfg.NCORES):
        m = {
            "y2": y2_full,
            "b2r": consts["b2r"],
            "iota": consts["iota"],
            "degP": degP[ci],
            "IDX": ep["IDX"][ci],
            "DLOC": ep["DLOC"][ci],
        }
        in_B.append(m)
    resB = runner(ncB, in_B)
    out = np.concatenate([r["outs"] for r in resB], axis=0)  # [NPAD, C_OUT]
    return out[: cfg.N]


def kernel(x, edge_index, W1, b1, W2, b2):
    return run(x, edge_index, W1, b1, W2, b2, FULL)


# revision 9
# speedup vs baseline: 1.5590x; 1.2522x over previous
"""GCN (2-layer, PyG GCNConv-style) on 8 Trainium2 NeuronCores via Bass/Tile.

Strategy:
  out = dinv * (A_sum @ y) + b per layer, with y = (x*dinv) @ W a node table.
  - dst nodes are split into 128-node blocks, blocks distributed over cores.
  - Edges grouped per (core, block, src-chunk); per 128-edge tile we
    dma_gather the source rows (256B each) and reduce with a one-hot matmul
    accumulating in PSUM (the segment-sum runs on the PE at full rate).
  - src-chunks of 32768 rows because dma_gather indices are int16; the four
    chunk gathers run on the four SWDGE queues in parallel (Q7 descriptor
    generation is the critical resource).
  - Two launches: A = y1 build + layer-1 aggregation -> y2 table shards;
    host concatenates shards (pure data movement); B = layer-2 aggregation.
  - All heavy float math happens on device; the host only does integer
    graph preprocessing (edge sorting/partitioning) and dtype casts.
"""

import numpy as np
import ml_dtypes

import concourse.bacc as bacc
import concourse.mybir as mybir
import concourse.tile as tile
from concourse.bass_utils import run_bass_kernel_spmd

BF16 = ml_dtypes.bfloat16
P = 128

# set by test.py to collect hardware profiles
TRACE = False
# emit pad-zeroing memsets (needed only to satisfy the simulator's
# uninitialized-read checker; the padded bytes are never used by compute)
SIM_SAFE = False
LAST_EXEC_NS = []


class Cfg:
    def __init__(self, N, F_IN, HID, C_OUT, NCORES, BPC, CHUNK, SBB, SLABW):
        self.N = N
        self.F_IN = F_IN          # must be 128 (one partition load)
        self.HID = HID
        self.C_OUT = C_OUT
        self.NCORES = NCORES
        self.BPC = BPC            # dst blocks per core
        self.CHUNK = CHUNK        # gather table chunk rows (int16 reach)
        self.SBB = SBB            # blocks per superblock (gather batching)
        self.SLABW = SLABW        # xT slab width for phase 1
        self.NB = NCORES * BPC    # total blocks
        self.NPAD = self.NB * P
        assert self.NPAD >= N
        self.NCHUNKS = -(-self.NPAD // CHUNK)
        assert self.NPAD % SLABW == 0
        self.NSLAB = self.NPAD // SLABW
        self.chunk_rows = [
            min(CHUNK, self.NPAD - c * CHUNK) for c in range(self.NCHUNKS)
        ]
        # superblock layout: BPC = full SBs of SBB blocks + possibly one partial
        self.sb_sizes = []
        left = BPC
        while left > 0:
            s = min(SBB, left)
            self.sb_sizes.append(s)
            left -= s
        self.NSB = len(self.sb_sizes)


FULL = Cfg(N=100000, F_IN=128, HID=64, C_OUT=40, NCORES=8, BPC=98,
           CHUNK=32768, SBB=8, SLABW=2048)


# --------------------------------------------------------------------------
# host-side integer preprocessing
# --------------------------------------------------------------------------

def prep_edges(src, dst, cfg):
    """Group edges by (core, block, chunk); emit per-core gather-index and
    dst-local streams plus the (uniform) per-chunk tile budgets."""
    NC, BPC, CH, SBB = cfg.NCORES, cfg.BPC, cfg.CHUNK, cfg.SBB
    nch = cfg.NCHUNKS
    core = dst // (BPC * P)
    blk = (dst % (BPC * P)) // P
    dloc = (dst % P).astype(np.int64)
    chunk = src // CH

    key = ((core * BPC + blk) * nch + chunk).astype(np.int64)
    order = np.argsort(key, kind="stable")
    skey = key[order]
    ssrc = src[order]
    sdl = dloc[order]

    nkeys = NC * BPC * nch
    counts = np.bincount(skey, minlength=nkeys)
    starts = np.zeros(nkeys + 1, np.int64)
    np.cumsum(counts, out=starts[1:])

    cnt3 = counts.reshape(NC, BPC, nch)
    T = [int(-(-cnt3[:, :, c].max() // P)) for c in range(nch)]
    T = [max(t, 1) for t in T]
    TT = int(sum(T))

    NG = cfg.NSB * nch
    nimax = SBB * max(T) * P
    IDX = np.zeros((NC, NG, P, nimax // 16), np.int16)
    DLOC = np.full((NC, cfg.NSB, P, SBB * TT), -1.0, BF16)
    offc = np.concatenate([[0], np.cumsum(T)]).astype(np.int64)

    for ci in range(NC):
        for sb in range(cfg.NSB):
            nblk = cfg.sb_sizes[sb]
            b0 = sum(cfg.sb_sizes[:sb])
            for c in range(nch):
                ni = nblk * T[c] * P
                idxs = np.zeros(ni, np.int64)
                dls = np.full(ni, -1, np.int64)
                for bl in range(nblk):
                    k = (ci * BPC + (b0 + bl)) * nch + c
                    s, n = starts[k], counts[k]
                    o = bl * T[c] * P
                    idxs[o:o + n] = ssrc[s:s + n] - c * CH
                    dls[o:o + n] = sdl[s:s + n]
                g = sb * nch + c
                wrapped = idxs.astype(np.int16).reshape(-1, 16).T  # [16, ni/16]
                IDX[ci, g, :, : ni // 16] = np.tile(wrapped, (8, 1))
                # dloc columns: [b_local, c, tt] -> col b_local*TT + offc[c] + tt
                dv = dls.reshape(nblk, T[c], P)
                for bl in range(nblk):
                    col0 = bl * TT + offc[c]
                    DLOC[ci, sb, :, col0:col0 + T[c]] = dv[bl].T.astype(BF16)
    return {"T": T, "TT": TT, "IDX": IDX, "DLOC": DLOC, "nimax": nimax}


def host_prep(x, edge_index, W1, b1, W2, b2, cfg):
    N = cfg.N
    loops = np.arange(N, dtype=np.int64)
    src = np.concatenate([edge_index[0].astype(np.int64), loops])
    dst = np.concatenate([edge_index[1].astype(np.int64), loops])

    deg = np.bincount(dst, minlength=cfg.NPAD).astype(np.float32)
    deg[deg == 0] = 1.0  # pad / isolated: dinv value never observed in output

    ep = prep_edges(src, dst, cfg)

    # xT tiled into slabs [NSLAB, 128, SLABW] bf16
    xT = np.zeros((cfg.F_IN, cfg.NPAD), np.float32)
    xT[:, :N] = x.T
    xTt = np.ascontiguousarray(
        xT.reshape(cfg.F_IN, cfg.NSLAB, cfg.SLABW).transpose(1, 0, 2)
    ).astype(BF16)

    degN = np.ascontiguousarray(deg.reshape(cfg.NB, P).T)          # [128, NB]
    degP = np.stack([degN[:, i * cfg.BPC:(i + 1) * cfg.BPC] for i in range(cfg.NCORES)])

    iota = np.broadcast_to(np.arange(P, dtype=BF16), (P, P)).copy()
    ident = np.eye(P, dtype=BF16)

    consts = {
        "xTt": xTt,
        "W1": W1.astype(BF16),
        "b1r": np.broadcast_to(b1.astype(np.float32), (P, cfg.HID)).copy(),
        "W2": W2.astype(BF16),
        "b2r": np.broadcast_to(b2.astype(np.float32), (P, cfg.C_OUT)).copy(),
        "degN": degN,
        "iota": iota,
        "ident": ident,
    }
    return ep, consts, degP


# --------------------------------------------------------------------------
# device programs
# --------------------------------------------------------------------------

def _dinv_tile(nc, cp, t_deg, cols):
    deg_t = cp.tile([P, cols], mybir.dt.float32)
    nc.sync.dma_start(out=deg_t[:], in_=t_deg[:, :])
    sq = cp.tile([P, cols], mybir.dt.float32)
    nc.scalar.activation(out=sq[:], in_=deg_t[:],
                         func=mybir.ActivationFunctionType.Sqrt)
    dinv = cp.tile([P, cols], mybir.dt.float32)
    nc.vector.reciprocal(out=dinv[:], in_=sq[:])
    return dinv


def _gather_phase(nc, tc, cfg, ep, chunk_ap, iota_t, body, gbufs):
    """Shared gather/aggregate skeleton. body(bl_glob, matmul_feeder);
    matmul_feeder(ph, rhs_w) issues the TT one-hot matmuls into psum ph."""
    T, TT = ep["T"], ep["TT"]
    with (
        tc.tile_pool(name="gpool", bufs=gbufs) as gp,
        tc.tile_pool(name="ohpool", bufs=3) as ohp,
    ):
        for sb in range(cfg.NSB):
            nblk = cfg.sb_sizes[sb]
            b0 = sum(cfg.sb_sizes[:sb])
            g_ts = []
            for c in range(cfg.NCHUNKS):
                ni = nblk * T[c] * P
                g = sb * cfg.NCHUNKS + c
                idx_t = gp.tile([P, ni // 16], mybir.dt.int16, tag=f"idx{c}")
                nc.sync.dma_start(out=idx_t[:], in_=nc.t_IDX[g, :, : ni // 16])
                gt = gp.tile([P, nblk * T[c], P], mybir.dt.bfloat16, tag=f"g{c}")
                nc.gpsimd.dma_gather(
                    out_ap=gt[:],
                    in_ap=chunk_ap(c),
                    idxs_ap=idx_t[:],
                    num_idxs=ni,
                    num_idxs_reg=ni,
                    elem_size=P,
                    single_packet=False,
                    queue_num=c % 4,
                )
                g_ts.append(gt)
            dloc_t = gp.tile([P, cfg.SBB * TT], mybir.dt.bfloat16, tag="dloc")
            nc.sync.dma_start(
                out=dloc_t[:, : nblk * TT], in_=nc.t_DLOC[sb, :, : nblk * TT]
            )

            for bl in range(nblk):
                # all TT one-hots for this block in one DVE op
                ohb = ohp.tile([P, TT, P], mybir.dt.bfloat16, tag="oh")
                nc.vector.tensor_tensor(
                    out=ohb[:],
                    in0=dloc_t[:, bl * TT:(bl + 1) * TT]
                        .unsqueeze(2).to_broadcast([P, TT, P]),
                    in1=iota_t[:].unsqueeze(1).to_broadcast([P, TT, P]),
                    op=mybir.AluOpType.is_equal,
                )

                def feeder(ph, rhs_w, ohb=ohb, bl=bl, g_ts=g_ts):
                    k = 0
                    for c in range(cfg.NCHUNKS):
                        for tt in range(T[c]):
                            nc.tensor.matmul(
                                out=ph[:],
                                lhsT=ohb[:, k, :],
                                rhs=g_ts[c][:, bl * T[c] + tt, 0:rhs_w],
                                start=(k == 0),
                                stop=(k == TT - 1),
                            )
                            k += 1
                body(b0 + bl, feeder)


def build_launch_A(cfg, ep):
    nc = bacc.Bacc(None, target_bir_lowering=False, name="gcn_a",
                   num_swdge_queues=4)
    HID, COUT = cfg.HID, cfg.C_OUT
    t_xTt = nc.dram_tensor("xTt", [cfg.NSLAB, P, cfg.SLABW], mybir.dt.bfloat16, kind="ExternalInput")
    t_W1 = nc.dram_tensor("W1", [cfg.F_IN, HID], mybir.dt.bfloat16, kind="ExternalInput")
    t_b1r = nc.dram_tensor("b1r", [P, HID], mybir.dt.float32, kind="ExternalInput")
    t_W2 = nc.dram_tensor("W2", [HID, COUT], mybir.dt.bfloat16, kind="ExternalInput")
    t_degN = nc.dram_tensor("degN", [P, cfg.NB], mybir.dt.float32, kind="ExternalInput")
    t_degP = nc.dram_tensor("degP", [P, cfg.BPC], mybir.dt.float32, kind="ExternalInput")
    t_iota = nc.dram_tensor("iota", [P, P], mybir.dt.bfloat16, kind="ExternalInput")
    t_ident = nc.dram_tensor("ident", [P, P], mybir.dt.bfloat16, kind="ExternalInput")
    nc.t_IDX = nc.dram_tensor("IDX", list(ep["IDX"].shape[1:]), mybir.dt.int16, kind="ExternalInput")
    nc.t_DLOC = nc.dram_tensor("DLOC", list(ep["DLOC"].shape[1:]), mybir.dt.bfloat16, kind="ExternalInput")
    t_y2s = nc.dram_tensor("y2s", [cfg.BPC * P, P], mybir.dt.bfloat16, kind="ExternalOutput")

    WG = 4 if (cfg.SLABW // P) % 4 == 0 else 1  # node-chunks per y1-write DMA

    with tile.TileContext(nc) as tc:
        with (
            tc.tile_pool(name="consts", bufs=1) as cp,
            tc.tile_pool(name="dram", bufs=1, space="DRAM") as dp,
        ):
            # per-chunk y1 tables so gathers can start while later chunks build
            y1c = [dp.tile([cfg.chunk_rows[c], P], mybir.dt.bfloat16,
                           name=f"y1c{c}", tag=f"y1c{c}")
                   for c in range(cfg.NCHUNKS)]
            w1_t = cp.tile([cfg.F_IN, HID], mybir.dt.bfloat16)
            nc.sync.dma_start(out=w1_t[:], in_=t_W1[:, :])
            w2_t = cp.tile([HID, COUT], mybir.dt.bfloat16)
            nc.sync.dma_start(out=w2_t[:], in_=t_W2[:, :])
            b1r_t = cp.tile([P, HID], mybir.dt.float32)
            nc.sync.dma_start(out=b1r_t[:], in_=t_b1r[:, :])
            iota_t = cp.tile([P, P], mybir.dt.bfloat16)
            nc.sync.dma_start(out=iota_t[:], in_=t_iota[:, :])
            ident_t = cp.tile([P, P], mybir.dt.bfloat16)
            nc.sync.dma_start(out=ident_t[:], in_=t_ident[:, :])

            dinvN = _dinv_tile(nc, cp, t_degN, cfg.NB)
            dinvP = _dinv_tile(nc, cp, t_degP, cfg.BPC)

            # phase 1: y1 = (x @ W1) * dinv  (bf16 rows padded to 128 elems)
            npc = cfg.SLABW // P  # node chunks per slab
            assert npc % WG == 0
            slab_order = [cfg.NSLAB - 1] + list(range(cfg.NSLAB - 1))
            with (
                tc.tile_pool(name="ph1", bufs=3) as p1,
                tc.tile_pool(name="ph1ps", bufs=2, space="PSUM") as p1p,
            ):
                for s in slab_order:
                    slab = p1.tile([P, cfg.SLABW], mybir.dt.bfloat16, tag="slab")
                    nc.sync.dma_start(out=slab[:], in_=t_xTt[s, :, :])
                    for j4 in range(npc // WG):
                        row4 = p1.tile([P, WG, P], mybir.dt.bfloat16, tag="row")
                        for k in range(WG):
                            j = j4 * WG + k
                            jj = s * npc + j
                            ps = p1p.tile([P, HID], mybir.dt.float32, tag="psy")
                            nc.tensor.matmul(
                                out=ps[:], lhsT=slab[:, j * P:(j + 1) * P],
                                rhs=w1_t[:], start=True, stop=True,
                            )
                            if SIM_SAFE:
                                nc.vector.memset(row4[:, k, HID:], 0)
                            nc.vector.tensor_tensor(
                                out=row4[:, k, :HID], in0=ps[:],
                                in1=dinvN[:, jj:jj + 1].to_broadcast([P, HID]),
                                op=mybir.AluOpType.mult,
                            )
                        jj0 = s * npc + j4 * WG
                        cc = (jj0 * P) // cfg.CHUNK
                        lo = jj0 * P - cc * cfg.CHUNK
                        nc.scalar.dma_start(
                            out=y1c[cc][lo:lo + WG * P, :]
                                .rearrange("(k p) f -> p k f", p=P),
                            in_=row4[:],
                        )

            # phase 2: layer-1 aggregation + y2 table rows
            with (
                tc.tile_pool(name="ep1", bufs=3) as e1,
                tc.tile_pool(name="hps", bufs=2, space="PSUM") as hps,
                tc.tile_pool(name="tps", bufs=2, space="PSUM") as tps,
                tc.tile_pool(name="yps", bufs=2, space="PSUM") as yps,
            ):
                def body(bg, feeder):
                    ph = hps.tile([P, HID], mybir.dt.float32, tag="ph")
                    feeder(ph, HID)
                    dv = dinvP[:, bg:bg + 1]
                    t1 = e1.tile([P, HID], mybir.dt.float32, tag="t1")
                    nc.scalar.activation(
                        out=t1[:], in_=ph[:],
                        func=mybir.ActivationFunctionType.Copy, scale=dv)
                    t2 = e1.tile([P, HID], mybir.dt.float32, tag="t2")
                    nc.vector.tensor_tensor(
                        out=t2[:], in0=t1[:], in1=b1r_t[:], op=mybir.AluOpType.add,
                    )
                    hd = e1.tile([P, HID], mybir.dt.bfloat16, tag="hd")
                    nc.scalar.activation(
                        out=hd[:], in_=t2[:],
                        func=mybir.ActivationFunctionType.Relu, scale=dv,
                    )
                    ptr = tps.tile([HID, P], mybir.dt.bfloat16, tag="ptr")
                    nc.tensor.transpose(out=ptr[:], in_=hd[:], identity=ident_t[:])
                    hdT = e1.tile([HID, P], mybir.dt.bfloat16, tag="hdT")
                    nc.vector.tensor_copy(out=hdT[:], in_=ptr[:])
                    py2 = yps.tile([P, COUT], mybir.dt.float32, tag="py2")
                    nc.tensor.matmul(out=py2[:], lhsT=hdT[:], rhs=w2_t[:], start=True, stop=True)
                    yrow = e1.tile([P, P], mybir.dt.bfloat16, tag="yrow")
                    if SIM_SAFE:
                        nc.vector.memset(yrow[:, COUT:], 0)
                    nc.vector.tensor_copy(out=yrow[:, :COUT], in_=py2[:])
                    nc.sync.dma_start(out=t_y2s[bg * P:(bg + 1) * P, :], in_=yrow[:])

                _gather_phase(nc, tc, cfg, ep, lambda c: y1c[c][:, :],
                              iota_t, body, gbufs=2)
    nc.compile()
    return nc


def build_launch_B(cfg, ep):
    nc = bacc.Bacc(None, target_bir_lowering=False, name="gcn_b",
                   num_swdge_queues=4)
    COUT = cfg.C_OUT
    t_y2 = nc.dram_tensor("y2", [cfg.NPAD, P], mybir.dt.bfloat16, kind="ExternalInput")
    t_b2r = nc.dram_tensor("b2r", [P, COUT], mybir.dt.float32, kind="ExternalInput")
    t_degP = nc.dram_tensor("degP", [P, cfg.BPC], mybir.dt.float32, kind="ExternalInput")
    t_iota = nc.dram_tensor("iota", [P, P], mybir.dt.bfloat16, kind="ExternalInput")
    nc.t_IDX = nc.dram_tensor("IDX", list(ep["IDX"].shape[1:]), mybir.dt.int16, kind="ExternalInput")
    nc.t_DLOC = nc.dram_tensor("DLOC", list(ep["DLOC"].shape[1:]), mybir.dt.bfloat16, kind="ExternalInput")
    t_out = nc.dram_tensor("outs", [cfg.BPC * P, COUT], mybir.dt.float32, kind="ExternalOutput")

    with tile.TileContext(nc) as tc:
        with tc.tile_pool(name="consts", bufs=1) as cp:
            iota_t = cp.tile([P, P], mybir.dt.bfloat16)
            nc.sync.dma_start(out=iota_t[:], in_=t_iota[:, :])
            b2r_t = cp.tile([P, COUT], mybir.dt.float32)
            nc.sync.dma_start(out=b2r_t[:], in_=t_b2r[:, :])
            dinvP = _dinv_tile(nc, cp, t_degP, cfg.BPC)

            with (
                tc.tile_pool(name="ep2", bufs=3) as e2,
                tc.tile_pool(name="ops", bufs=2, space="PSUM") as ops,
            ):
                def body(bg, feeder):
                    po = ops.tile([P, COUT], mybir.dt.float32, tag="po")
                    feeder(po, COUT)
                    t1 = e2.tile([P, COUT], mybir.dt.float32, tag="t1")
                    nc.scalar.activation(
                        out=t1[:], in_=po[:],
                        func=mybir.ActivationFunctionType.Copy,
                        scale=dinvP[:, bg:bg + 1])
                    ot = e2.tile([P, COUT], mybir.dt.float32, tag="ot")
                    nc.vector.tensor_tensor(
                        out=ot[:], in0=t1[:], in1=b2r_t[:], op=mybir.AluOpType.add,
                    )
                    nc.sync.dma_start(out=t_out[bg * P:(bg + 1) * P, :], in_=ot[:])

                def chunk_ap(c):
                    lo = c * cfg.CHUNK
                    return t_y2[lo:lo + cfg.chunk_rows[c], :]

                _gather_phase(nc, tc, cfg, ep, chunk_ap, iota_t, body, gbufs=3)
    nc.compile()
    return nc


# --------------------------------------------------------------------------
# entry point
# --------------------------------------------------------------------------

def run(x, edge_index, W1, b1, W2, b2, cfg, runner=None):
    global LAST_EXEC_NS
    LAST_EXEC_NS = []
    ep, consts, degP = host_prep(
        np.asarray(x, np.float32), np.asarray(edge_index), np.asarray(W1),
        np.asarray(b1), np.asarray(W2), np.asarray(b2), cfg)

    ncA = build_launch_A(cfg, ep)
    ncB = build_launch_B(cfg, ep)

    in_A = []
    for ci in range(cfg.NCORES):
        m = {k: consts[k] for k in
             ("xTt", "W1", "b1r", "W2", "degN", "iota", "ident")}
        m["degP"] = degP[ci]
        m["IDX"] = ep["IDX"][ci]
        m["DLOC"] = ep["DLOC"][ci]
        in_A.append(m)

    if runner is None:
        def runner(nc, in_maps):
            res = run_bass_kernel_spmd(
                nc, in_maps, core_ids=list(range(cfg.NCORES)), trace=TRACE)
            LAST_EXEC_NS.append(res.exec_time_ns)
            return res.results

    resA = runner(ncA, in_A)
    y2_full = np.concatenate([r["y2s"] for r in resA], axis=0)  # [NPAD, 128]

    in_B = []
    for ci in range(cfg.NCORES):
        m = {
            "y2": y2_full,
            "b2r": consts["b2r"],
            "iota": consts["iota"],
            "degP": degP[ci],
            "IDX": ep["IDX"][ci],
            "DLOC": ep["DLOC"][ci],
        }
        in_B.append(m)
    resB = runner(ncB, in_B)
    out = np.concatenate([r["outs"] for r in resB], axis=0)  # [NPAD, C_OUT]
    return out[: cfg.N]


def kernel(x, edge_index, W1, b1, W2, b2):
    return run(x, edge_index, W1, b1, W2, b2, FULL)
